# revision 31
# baseline (speedup 1.0000x reference)
# Trainium2 Bass kernel for nn_FHNTritonAttention: causal attention with an
# FHN (FitzHugh-Nagumo) gate on the attention probabilities.
#
# Math note that shapes the whole kernel: attn_energy = softmax(scores).sum(-1)
# is ~1.0 for every row (softmax rows sum to 1), so stimulus_normed == 1,
# threshold_gate == sigmoid(5), and the FHN recurrence collapses to one
# constant per run. The gate multiplies each probability row by a constant c
# and the subsequent renormalization divides it back out:
#   p'' = p*c / (c*S + 1e-8) = p / (S + 1e-8/c),  S = row sum ~= 1.
# So the entire FHN block reduces to scaling the output by
# f0 = 1/(1 + 1e-8/c0), computed on host from (a, b, dt) and folded into Wout.
# The deviations this ignores are O(1e-7) relative — far below fp32 matmul
# noise of the reference itself.
#
# Device kernel (SPMD over 8 cores; core = (batch, 4-head group)):
#   The whole kernel is emitted as ONE software-pipelined stream so the PE
#   never stalls on the softmax (scalar engine) dependency chain:
#     - A(n) blocks: qkT = Wqk_slice @ x.T (feature-on-partition layout) and
#       v (natural layout + ones column for the softmax denominator Z).
#     - B(g) chunks: per head pair, per 128-deep s chunk: scoresT for both
#       heads into one [128,1024] psum, one strided exp (scalar), causal-mask
#       multiply on diagonal chunks (gpsimd), PV accumulate (with ones column
#       producing Z). PV for chunk j is emitted one chunk late so the PE
#       in-order queue never waits on exp.
#     - OP(g) blocks: out-projection per q tile, evict + DMA out.
#   Schedule: A(0); then for each g: B(g) chunks interleaved with A(g+1) and
#   OP(g-1) blocks as PE filler. Evictions/normalize are spread across
#   scalar (head-0 evict, qk evict), DVE (recip, head-1 evict, normalize,
#   v/y evict) and gpsimd (1/Z partition-broadcast, masks) so no engine
#   exceeds the PE's total work.
#
# Matmuls run in bf16 (inputs pre-cast on host; fp32 PSUM accumulation).

import math
import os

import numpy as np

N_HEADS = 16
HEAD_DIM = 64
THRESHOLD = 0.5
TAU = 12.5
N_FHN_STEPS = 2

N_CORES = 8
HEADS_PER_CORE = 4  # cores 0-3 -> batch 0, cores 4-7 -> batch 1

ATTN_DTYPE = os.environ.get("KERNEL_ATTN_DTYPE", "bfloat16")

LAST_RUN = {}  # filled with exec_time_ns etc. when KERNEL_TRACE is set

_PROGRAM_CACHE = {}


def _fhn_scale(a, b, dt):
    """Host-side replica of the reference's gate math at attn_energy == 1."""
    a = float(a)
    b = float(b)
    dt = float(dt)
    sig5 = 1.0 / (1.0 + math.exp(-(1.0 - THRESHOLD) * 10.0))
    i0 = 1.0 * (0.1 + 0.9 * sig5)
    v = 0.0
    w = 0.0
    for _ in range(N_FHN_STEPS):
        v = v + dt * (v - v**3 / 3.0 - w + i0)
        w = (w + (dt / TAU) * (v + a)) / (1.0 + (dt / TAU) * b)
    gate = 1.0 / (1.0 + math.exp(-v))
    c0 = 0.5 + 0.5 * gate
    return c0 / (c0 + 1e-8)


def _build_program(T, D, H_per_core, hd):
    import concourse.mybir as mybir
    import concourse.tile as tile
    from concourse import bacc

    f32 = mybir.dt.float32
    at_dt = getattr(mybir.dt, ATTN_DTYPE)
    P = 128
    QT = 512   # q tile width (free dim of score/PV matmuls)
    SC = 128   # s chunk depth (contraction of PV, partitions of scoresT)
    K_D = D // P
    QK_ROWS = 2 * H_per_core * hd
    V_COLS = H_per_core * hd
    C = H_per_core * hd
    n_qt = T // QT
    n_pairs = H_per_core // 2
    inv_sqrt = 1.0 / math.sqrt(hd)

    nc = bacc.Bacc("TRN2", target_bir_lowering=False, debug=False,
                   num_devices=N_CORES)

    # all inputs pre-packed on host to [128, X] (partition-major tiles side
    # by side in the free dim) so each loads with ONE dma_start whose row
    # descriptors fan out across all 16 DMA engines.
    xt_d = nc.declare_dram_parameter("xt", [P, K_D * T], at_dt, isOutput=False)
    wqkt_d = nc.declare_dram_parameter("wqkt", [P, K_D * QK_ROWS], at_dt,
                                       isOutput=False)
    wvt_d = nc.declare_dram_parameter("wvt", [P, K_D * V_COLS], at_dt,
                                      isOutput=False)
    wot_d = nc.declare_dram_parameter("wot", [P, (C // P) * D], at_dt,
                                      isOutput=False)
    masks_d = nc.declare_dram_parameter("masks", [P, 4 * 2 * QT], at_dt,
                                        isOutput=False)
    yt_d = nc.declare_dram_parameter("yt", [D, T], at_dt, isOutput=True)

    yt_t = yt_d.rearrange("(a p) t -> a p t", p=P)

    with nc.allow_low_precision(reason="bf16/f32r compute is intentional"), \
            tile.TileContext(nc) as tc:
        with (
            tc.tile_pool(name="persist", bufs=1) as persist,
            tc.tile_pool(name="aps_pool", bufs=2, space="PSUM") as aps_pool,
            tc.tile_pool(name="sc_ps", bufs=2, space="PSUM") as sc_ps,
            tc.tile_pool(name="pv_ps", bufs=1, space="PSUM") as pv_ps,
            tc.tile_pool(name="u_sb", bufs=6) as u_pool,
            tc.tile_pool(name="norm", bufs=1) as norm_pool,
            tc.tile_pool(name="y_sb", bufs=6) as y_pool,
        ):
            # ---- persistent SBUF tiles ----
            xt_all = persist.tile([P, K_D * T], at_dt, name="xt_all",
                                  tag="xt_all")
            wqkt_all = persist.tile([P, K_D * QK_ROWS], at_dt,
                                    name="wqkt_all", tag="wqkt_all")
            wvt_all = persist.tile([P, K_D * V_COLS], at_dt, name="wvt_all",
                                   tag="wvt_all")
            xt = [xt_all[:, k * T:(k + 1) * T] for k in range(K_D)]
            wqkt = [wqkt_all[:, k * QK_ROWS:(k + 1) * QK_ROWS]
                    for k in range(K_D)]
            wvt = [wvt_all[:, k * V_COLS:(k + 1) * V_COLS]
                   for k in range(K_D)]
            masks = persist.tile([P, 8 * QT], at_dt, name="masks", tag="masks")
            wot_all = persist.tile([P, (C // P) * D], at_dt, name="wot_all",
                                   tag="wot_all")
            wot = [wot_all[:, k * D:(k + 1) * D] for k in range(C // P)]
            qkt = [persist.tile([P, T], at_dt, name=f"qkt{m}", tag=f"qkt{m}")
                   for m in range(QK_ROWS // P)]
            v_pad = [persist.tile([P, H_per_core * (hd + 1)], at_dt,
                                  name=f"vp{m}", tag=f"vp{m}")
                     for m in range(T // P)]
            attn = [persist.tile([P, T], at_dt, name=f"attn{p}", tag=f"attn{p}")
                    for p in range(n_pairs)]

            # ---- input DMAs: submission spread across engine queues so the
            # descriptor generation (~0.6us per dma_start) does not serialize
            # the input stream on one sequencer.
            nc.sync.dma_start(out=wqkt_all[:], in_=wqkt_d[:])
            xt_n0_sb = xt_all.rearrange("p (k t) -> p k t", k=K_D)[:, :, 0:QT]
            xt_n0_dr = xt_d.rearrange("p (k t) -> p k t", k=K_D)[:, :, 0:QT]
            nc.gpsimd.dma_start(out=xt_n0_sb, in_=xt_n0_dr)
            nc.scalar.dma_start(out=wvt_all[:], in_=wvt_d[:])
            nc.scalar.dma_start(out=masks[:], in_=masks_d[:])
            nc.sync.dma_start(out=wot_all[:], in_=wot_d[:])
            xt_nr_sb = xt_all.rearrange("p (k t) -> p k t", k=K_D)[:, :, QT:]
            xt_nr_dr = xt_d.rearrange("p (k t) -> p k t", k=K_D)[:, :, QT:]
            nc.gpsimd.dma_start(out=xt_nr_sb, in_=xt_nr_dr)

            # ones columns of v_pad (persist; set once)
            for m in range(T // P):
                ones_cols = v_pad[m].rearrange(
                    "p (h x) -> p h x", x=hd + 1)[:, :, hd:]
                nc.vector.memset(ones_cols, 1.0)

            # ---- emission units ----
            def emit_qk_group(n, m):
                ps = aps_pool.tile([P, QT], f32, name="qkps", tag="aps")
                for k in range(K_D):
                    nc.tensor.matmul(
                        ps[:],
                        lhsT=wqkt[k][:, m * P:(m + 1) * P],
                        rhs=xt[k][:, n * QT:(n + 1) * QT],
                        start=(k == 0), stop=(k == K_D - 1),
                    )
                nc.scalar.copy(qkt[m][:, n * QT:(n + 1) * QT], ps[:])

            def emit_v_group(n, i):
                m = 4 * n + i
                ps = aps_pool.tile([P, V_COLS], f32, name="vps", tag="aps")
                for k in range(K_D):
                    nc.tensor.matmul(
                        ps[:],
                        lhsT=xt[k][:, m * P:(m + 1) * P],
                        rhs=wvt[k][:],
                        start=(k == 0), stop=(k == K_D - 1),
                    )
                nc.vector.tensor_copy(
                    v_pad[m].rearrange("p (h x) -> p h x", x=hd + 1)[:, :, 0:hd],
                    ps.rearrange("p (h x) -> p h x", x=hd),
                )

            # per-(g,p) attention state carried across chunk units
            class PairState:
                pass

            def emit_chunk_front(st, g, p, j):
                """scores + exp + mask for chunk j (PV deferred one chunk)."""
                q0 = g * QT
                s0 = j * SC
                r = (s0 - q0) // SC
                w0 = max(r, 0) * SC
                sc = sc_ps.tile([P, 2 * QT], f32, name="sc", tag="sc")
                qT = qkt[p]
                kT = qkt[n_pairs + p]
                for e in range(2):
                    lo, hi = e * hd, e * hd + hd
                    nc.tensor.matmul(
                        sc[:, e * QT + w0:(e + 1) * QT],
                        lhsT=kT[lo:hi, s0:s0 + SC],
                        rhs=qT[lo:hi, q0 + w0:q0 + QT],
                        start=True, stop=True,
                    )
                u = u_pool.tile([P, 2 * QT], at_dt, name="u", tag="u")
                if w0 == 0:
                    nc.scalar.activation(
                        u[:], sc[:], mybir.ActivationFunctionType.Exp,
                        scale=inv_sqrt,
                    )
                else:
                    u_v = u.rearrange("p (e t) -> p e t", e=2)
                    sc_v = sc.rearrange("p (e t) -> p e t", e=2)
                    nc.scalar.activation(
                        u_v[:, :, w0:], sc_v[:, :, w0:],
                        mybir.ActivationFunctionType.Exp, scale=inv_sqrt,
                    )
                if r >= 0:
                    u_v = u.rearrange("p (e t) -> p e t", e=2)
                    m_v = masks[:, r * 2 * QT:(r + 1) * 2 * QT].rearrange(
                        "p (e t) -> p e t", e=2)
                    nc.vector.tensor_mul(
                        u_v[:, :, w0:w0 + SC], u_v[:, :, w0:w0 + SC],
                        m_v[:, :, w0:w0 + SC])
                st.pend.append((j, u, w0))

            def emit_chunk_pv(st, g, p):
                """PV for the oldest pending chunk."""
                j, u, w0 = st.pend.pop(0)
                n_sc = (g * QT + QT) // SC
                for e in range(2):
                    h = 2 * p + e
                    nc.tensor.matmul(
                        st.pv[0:hd + 1, e * QT + w0:(e + 1) * QT],
                        lhsT=v_pad[j][:, h * (hd + 1):(h + 1) * (hd + 1)],
                        rhs=u[:, e * QT + w0:(e + 1) * QT],
                        start=(j == 0), stop=(j == n_sc - 1),
                    )

            def emit_evict(st, g, p, final=False):
                """Drain pv psum: unnormalized outT (rows 0:64) + Z (row 64).

                final=True (last pair of the kernel, straight into OP(3)):
                split the Z chain per head so recip/broadcast/normalize
                pipeline instead of running serially on the critical path.
                """
                q0 = g * QT
                if final:
                    reps = []
                    for e in range(2):
                        zr = norm_pool.tile([1, QT], f32, name="zr",
                                            tag=f"zr{e}", bufs=1)
                        nc.vector.tensor_copy(
                            zr[0:1, :], st.pv[hd:hd + 1, e * QT:(e + 1) * QT])
                        rr = norm_pool.tile([1, QT], f32, name="rr",
                                            tag=f"rr{e}", bufs=1)
                        nc.vector.reciprocal_approx_fast(out=rr[0:1, :],
                                                         in_=zr[0:1, :])
                        rep = norm_pool.tile([P, QT], f32, name="repf",
                                             tag=f"repf{e}", bufs=1)
                        nc.gpsimd.partition_broadcast(rep[:], rr[0:1, :])
                        reps.append(rep)
                    # evicts after the Z chains are queued
                    nc.scalar.copy(attn[p][0:hd, q0:q0 + QT],
                                   st.pv[0:hd, 0:QT])
                    nc.vector.tensor_copy(
                        attn[p][hd:P, q0:q0 + QT], st.pv[0:hd, QT:2 * QT])
                    st.reps = reps
                    st.rep = None
                    return
                # head 0 evict on scalar (partition-aligned 0:64 -> 0:64)
                nc.scalar.copy(attn[p][0:hd, q0:q0 + QT], st.pv[0:hd, 0:QT])
                # Z row (both heads) to partition 0 of SBUF, then 1/Z
                zrow = norm_pool.tile([1, 2 * QT], f32, name="zrow",
                                      tag="zrow", bufs=2)
                nc.vector.tensor_copy(zrow[0:1, :], st.pv[hd:hd + 1, :])
                # head 1 evict on DVE (partition shift 0:64 -> 64:128)
                nc.vector.tensor_copy(
                    attn[p][hd:P, q0:q0 + QT], st.pv[0:hd, QT:2 * QT])
                rrow = norm_pool.tile([1, 2 * QT], f32, name="rrow",
                                      tag="rrow", bufs=2)
                nc.vector.reciprocal_approx_fast(out=rrow[0:1, :],
                                                 in_=zrow[0:1, :])
                rep = norm_pool.tile([P, 2 * QT], f32, name="rep",
                                     tag="rep", bufs=2)
                nc.gpsimd.partition_broadcast(rep[:], rrow[0:1, :])
                st.rep = rep

            def emit_norm(st, g, p):
                q0 = g * QT
                for e in range(2):
                    sl = attn[p][e * hd:(e + 1) * hd, q0:q0 + QT]
                    if st.rep is None:
                        rep_sl = st.reps[e][e * hd:(e + 1) * hd, :]
                    else:
                        rep_sl = st.rep[e * hd:(e + 1) * hd,
                                        e * QT:(e + 1) * QT]
                    nc.vector.tensor_mul(sl, sl, rep_sl)

            def emit_op_block(g, m, y_eng):
                ps = aps_pool.tile([P, QT], f32, name="yps", tag="aps")
                for k in range(C // P):
                    nc.tensor.matmul(
                        ps[:],
                        lhsT=wot[k][:, m * P:(m + 1) * P],
                        rhs=attn[k][:, g * QT:(g + 1) * QT],
                        start=(k == 0), stop=(k == C // P - 1),
                    )
                y = y_pool.tile([P, QT], at_dt, name="y", tag="y")
                if y_eng == "scalar":
                    nc.scalar.copy(y[:], ps[:])
                else:
                    nc.vector.tensor_copy(y[:], ps[:])
                nc.sync.dma_start(
                    out=yt_t[m][:, g * QT:(g + 1) * QT], in_=y[:])

            # ---- schedule ----
            # A(0): all q/k first (their DMAs land first), then v (wvt lands
            # a little later).
            for unit in [(0, 0), (0, 2), (0, 1), (0, 3)]:
                emit_qk_group(*unit)
            for i in range(4):
                emit_v_group(0, i)

            def fill_units(g):
                """PE filler for segment g: A(g+1) groups; OP blocks are
                back-loaded into the final segment (B(3) has the thinnest
                fill ratio and is exp-rate-balanced, so it needs the most
                independent PE work)."""
                units = []
                if g + 1 < n_qt:
                    a = [lambda n=g + 1, m=m: emit_qk_group(n, m)
                         for m in range(QK_ROWS // P)]
                    v = [lambda n=g + 1, i=i: emit_v_group(n, i)
                         for i in range(4)]
                    # alternate qk/v
                    for x, y in zip(a, v):
                        units += [x, y]
                ops = {1: [0], 3: [1, 2]}.get(g, [])
                for gg in ops:
                    units += [lambda gg=gg, m=m: emit_op_block(gg, m, "dve")
                              for m in range(D // P)]
                return units

            prev_norm = None  # pair-1 normalize deferred from previous seg
            for g in range(n_qt):
                n_sc = (g * QT + QT) // SC
                primary = []
                states = [PairState(), PairState()]
                if prev_norm is not None:
                    # MUST precede OP(g-1) fill units: deps follow emission
                    # order and OP reads the region this normalizes.
                    primary.append(prev_norm)
                for p in range(n_pairs):
                    st = states[p]
                    st.pend = []

                    def mk_alloc(st=st):
                        def start_pair():
                            st.pv = pv_ps.tile([hd + 1, 2 * QT], f32,
                                               name="pv", tag="pv")
                        return start_pair
                    primary.append(mk_alloc())
                    for j in range(n_sc):
                        primary.append(
                            lambda st=st, g=g, p=p, j=j:
                                emit_chunk_front(st, g, p, j))
                        if j > 0:
                            primary.append(
                                lambda st=st, g=g, p=p: emit_chunk_pv(st, g, p))
                        if p == 1 and j == 2:
                            # normalize pair 0 once its broadcast has drained
                            primary.append(
                                lambda st=states[0], g=g: emit_norm(st, g, 0))
                    primary.append(
                        lambda st=st, g=g, p=p: emit_chunk_pv(st, g, p))
                    fin = (g == n_qt - 1 and p == n_pairs - 1)
                    if fin:
                        # emit the last pair's evict AFTER all fill units so
                        # no fill work (DVE y-evicts) queues ahead of the
                        # final normalize chain.
                        final_evict = (
                            lambda st=st, g=g, p=p:
                                emit_evict(st, g, p, final=True))
                    else:
                        primary.append(
                            lambda st=st, g=g, p=p: emit_evict(st, g, p))
                # pair 1 normalize goes to the start of the next segment
                prev_norm = lambda st=states[1], g=g: emit_norm(st, g, 1)

                fills = fill_units(g)
                # interleave fills evenly between primary units
                out_plan = []
                f = 0
                for i, u in enumerate(primary):
                    out_plan.append(u)
                    want = (i + 1) * len(fills) // len(primary)
                    while f < want:
                        out_plan.append(fills[f])
                        f += 1
                for u in out_plan:
                    u()
                if g == n_qt - 1:
                    final_evict()

            # tail: last normalize + OP(3)
            prev_norm()
            for m in range(D // P):
                emit_op_block(n_qt - 1, m, "scalar" if m % 2 else "dve")

    nc.finalize()
    return nc


def _make_masks(QT=512, SC=128):
    """Doubled causal masks: [128, 4*2*QT]; block r holds the mask for
    relative offset r twice side by side (head A | head B)."""
    i = np.arange(SC)[:, None]
    j = np.arange(QT)[None, :]
    blocks = []
    for r in range(4):
        m = (i + r * SC <= j).astype(np.float32)
        blocks += [m, m]
    return np.concatenate(blocks, axis=1)


def _cast(arr, dtype_name):
    if dtype_name == "bfloat16":
        import ml_dtypes
        return np.ascontiguousarray(arr.astype(ml_dtypes.bfloat16))
    return np.ascontiguousarray(arr.astype(np.float32))


def kernel(x, Wqkv, Wout, a, b, dt):
    from concourse.bass_utils import run_bass_kernel_spmd

    x = np.asarray(x, dtype=np.float32)
    Wqkv = np.asarray(Wqkv, dtype=np.float32)
    Wout = np.asarray(Wout, dtype=np.float32)
    B, T, D = x.shape
    H, hd = N_HEADS, HEAD_DIM
    hpc = HEADS_PER_CORE
    cores_per_batch = H // hpc
    f0 = _fhn_scale(a, b, dt)

    key = (T, D, hpc, hd)
    if key not in _PROGRAM_CACHE:
        _PROGRAM_CACHE[key] = _build_program(*key)
    nc = _PROGRAM_CACHE[key]

    masks = _cast(_make_masks(), ATTN_DTYPE)

    def _pack(arr):
        """[K*128, X] -> [128, K*X]: partition-major tiles side by side."""
        K = arr.shape[0] // 128
        return np.ascontiguousarray(
            arr.reshape(K, 128, -1).transpose(1, 0, 2).reshape(128, -1))

    in_maps = []
    for c in range(N_CORES):
        bi = c // cores_per_batch
        heads = range((c % cores_per_batch) * hpc, (c % cores_per_batch) * hpc + hpc)
        q_rows = np.concatenate([np.arange(h * hd, (h + 1) * hd) for h in heads])
        xt = _pack(_cast(x[bi].T, ATTN_DTYPE))                   # (128, 8*T)
        wqk = np.concatenate([Wqkv[q_rows], Wqkv[D + q_rows]], axis=0)
        wqkt = _pack(_cast(wqk.T, ATTN_DTYPE))                   # (128, 8*512)
        wvt = _pack(_cast(Wqkv[2 * D + q_rows].T, ATTN_DTYPE))   # (128, 8*256)
        wo = (Wout[:, q_rows].astype(np.float64) * f0).astype(np.float32)
        wot = _pack(_cast(wo.T, ATTN_DTYPE))                     # (128, 2*D)
        in_maps.append({"xt": xt, "wqkt": wqkt, "wvt": wvt, "wot": wot,
                        "masks": masks})

    trace_dir = os.environ.get("KERNEL_TRACE", "")
    kwargs = {}
    if trace_dir:
        try:
            import antenv.axon_hooks  # noqa: F401
        except ImportError:
            # agent image lacks the hook module; install the ctypes shim
            try:
                import sys
                import types

                from trn_agent_boot.trn_boot import _ntff_profile_via_ctypes
                hook = _ntff_profile_via_ctypes("/opt/axon/libaxon_pjrt.so")
                mod = types.ModuleType("antenv.axon_hooks")
                mod.get_axon_ntff_profile_hook = lambda: hook
                sys.modules["antenv.axon_hooks"] = mod
            except Exception:
                trace_dir = ""
    if trace_dir:
        os.makedirs(trace_dir, exist_ok=True)
        kwargs = {"trace": True, "tmpdir": trace_dir}
    res = run_bass_kernel_spmd(nc, in_maps, list(range(N_CORES)), **kwargs)
    LAST_RUN["exec_time_ns"] = res.exec_time_ns
    LAST_RUN["profile_json"] = res.profile_json

    out = np.zeros((B, T, D), dtype=np.float32)
    for bi in range(B):
        acc = np.zeros((D, T), dtype=np.float32)
        for c in range(bi * cores_per_batch, (bi + 1) * cores_per_batch):
            acc += res.results[c]["yt"].astype(np.float32)
        out[bi] = acc.T
    return out


# revision 32
# speedup vs baseline: 1.0062x; 1.0062x over previous
# Trainium2 Bass kernel for nn_FHNTritonAttention: causal attention with an
# FHN (FitzHugh-Nagumo) gate on the attention probabilities.
#
# Math note that shapes the whole kernel: attn_energy = softmax(scores).sum(-1)
# is ~1.0 for every row (softmax rows sum to 1), so stimulus_normed == 1,
# threshold_gate == sigmoid(5), and the FHN recurrence collapses to one
# constant per run. The gate multiplies each probability row by a constant c
# and the subsequent renormalization divides it back out:
#   p'' = p*c / (c*S + 1e-8) = p / (S + 1e-8/c),  S = row sum ~= 1.
# So the entire FHN block reduces to scaling the output by
# f0 = 1/(1 + 1e-8/c0), computed on host from (a, b, dt) and folded into Wout.
# The deviations this ignores are O(1e-7) relative — far below fp32 matmul
# noise of the reference itself.
#
# Device kernel (SPMD over 8 cores; core = (batch, 4-head group)):
#   The whole kernel is emitted as ONE software-pipelined stream so the PE
#   never stalls on the softmax (scalar engine) dependency chain:
#     - A(n) blocks: qkT = Wqk_slice @ x.T (feature-on-partition layout) and
#       v (natural layout + ones column for the softmax denominator Z).
#     - B(g) chunks: per head pair, per 128-deep s chunk: scoresT for both
#       heads into one [128,1024] psum, one strided exp (scalar), causal-mask
#       multiply on diagonal chunks (gpsimd), PV accumulate (with ones column
#       producing Z). PV for chunk j is emitted one chunk late so the PE
#       in-order queue never waits on exp.
#     - OP(g) blocks: out-projection per q tile, evict + DMA out.
#   Schedule: A(0); then for each g: B(g) chunks interleaved with A(g+1) and
#   OP(g-1) blocks as PE filler. Evictions/normalize are spread across
#   scalar (head-0 evict, qk evict), DVE (recip, head-1 evict, normalize,
#   v/y evict) and gpsimd (1/Z partition-broadcast, masks) so no engine
#   exceeds the PE's total work.
#
# Matmuls run in bf16 (inputs pre-cast on host; fp32 PSUM accumulation).

import math
import os

import numpy as np

N_HEADS = 16
HEAD_DIM = 64
THRESHOLD = 0.5
TAU = 12.5
N_FHN_STEPS = 2

N_CORES = 8
HEADS_PER_CORE = 4  # cores 0-3 -> batch 0, cores 4-7 -> batch 1

ATTN_DTYPE = os.environ.get("KERNEL_ATTN_DTYPE", "bfloat16")

LAST_RUN = {}  # filled with exec_time_ns etc. when KERNEL_TRACE is set

_PROGRAM_CACHE = {}


def _fhn_scale(a, b, dt):
    """Host-side replica of the reference's gate math at attn_energy == 1."""
    a = float(a)
    b = float(b)
    dt = float(dt)
    sig5 = 1.0 / (1.0 + math.exp(-(1.0 - THRESHOLD) * 10.0))
    i0 = 1.0 * (0.1 + 0.9 * sig5)
    v = 0.0
    w = 0.0
    for _ in range(N_FHN_STEPS):
        v = v + dt * (v - v**3 / 3.0 - w + i0)
        w = (w + (dt / TAU) * (v + a)) / (1.0 + (dt / TAU) * b)
    gate = 1.0 / (1.0 + math.exp(-v))
    c0 = 0.5 + 0.5 * gate
    return c0 / (c0 + 1e-8)


def _build_program(T, D, H_per_core, hd):
    import concourse.mybir as mybir
    import concourse.tile as tile
    from concourse import bacc

    f32 = mybir.dt.float32
    at_dt = getattr(mybir.dt, ATTN_DTYPE)
    P = 128
    QT = 512   # q tile width (free dim of score/PV matmuls)
    SC = 128   # s chunk depth (contraction of PV, partitions of scoresT)
    K_D = D // P
    QK_ROWS = 2 * H_per_core * hd
    V_COLS = H_per_core * hd
    C = H_per_core * hd
    n_qt = T // QT
    n_pairs = H_per_core // 2
    inv_sqrt = 1.0 / math.sqrt(hd)

    nc = bacc.Bacc("TRN2", target_bir_lowering=False, debug=False,
                   num_devices=N_CORES)

    # all inputs pre-packed on host to [128, X] (partition-major tiles side
    # by side in the free dim) so each loads with ONE dma_start whose row
    # descriptors fan out across all 16 DMA engines.
    xt_d = nc.declare_dram_parameter("xt", [P, K_D * T], at_dt, isOutput=False)
    wqkt_d = nc.declare_dram_parameter("wqkt", [P, K_D * QK_ROWS], at_dt,
                                       isOutput=False)
    wvt_d = nc.declare_dram_parameter("wvt", [P, K_D * V_COLS], at_dt,
                                      isOutput=False)
    wot_d = nc.declare_dram_parameter("wot", [P, (C // P) * D], at_dt,
                                      isOutput=False)
    masks_d = nc.declare_dram_parameter("masks", [P, 4 * 2 * QT], at_dt,
                                        isOutput=False)
    yt_d = nc.declare_dram_parameter("yt", [D, T], at_dt, isOutput=True)

    yt_t = yt_d.rearrange("(a p) t -> a p t", p=P)

    with nc.allow_low_precision(reason="bf16/f32r compute is intentional"), \
            tile.TileContext(nc) as tc:
        with (
            tc.tile_pool(name="persist", bufs=1) as persist,
            tc.tile_pool(name="aps_pool", bufs=2, space="PSUM") as aps_pool,
            tc.tile_pool(name="sc_ps", bufs=2, space="PSUM") as sc_ps,
            tc.tile_pool(name="pv_ps", bufs=1, space="PSUM") as pv_ps,
            tc.tile_pool(name="u_sb", bufs=6) as u_pool,
            tc.tile_pool(name="norm", bufs=1) as norm_pool,
            tc.tile_pool(name="y_sb", bufs=6) as y_pool,
        ):
            # ---- persistent SBUF tiles ----
            xt_all = persist.tile([P, K_D * T], at_dt, name="xt_all",
                                  tag="xt_all")
            wqkt_all = persist.tile([P, K_D * QK_ROWS], at_dt,
                                    name="wqkt_all", tag="wqkt_all")
            wvt_all = persist.tile([P, K_D * V_COLS], at_dt, name="wvt_all",
                                   tag="wvt_all")
            xt = [xt_all[:, k * T:(k + 1) * T] for k in range(K_D)]
            wqkt = [wqkt_all[:, k * QK_ROWS:(k + 1) * QK_ROWS]
                    for k in range(K_D)]
            wvt = [wvt_all[:, k * V_COLS:(k + 1) * V_COLS]
                   for k in range(K_D)]
            masks = persist.tile([P, 8 * QT], at_dt, name="masks", tag="masks")
            wot_all = persist.tile([P, (C // P) * D], at_dt, name="wot_all",
                                   tag="wot_all")
            wot = [wot_all[:, k * D:(k + 1) * D] for k in range(C // P)]
            qkt = [persist.tile([P, T], at_dt, name=f"qkt{m}", tag=f"qkt{m}")
                   for m in range(QK_ROWS // P)]
            v_pad = [persist.tile([P, H_per_core * (hd + 1)], at_dt,
                                  name=f"vp{m}", tag=f"vp{m}")
                     for m in range(T // P)]
            attn = [persist.tile([P, T], at_dt, name=f"attn{p}", tag=f"attn{p}")
                    for p in range(n_pairs)]

            # ---- input DMAs: submission spread across engine queues so the
            # descriptor generation (~0.6us per dma_start) does not serialize
            # the input stream on one sequencer.
            nc.sync.dma_start(out=wqkt_all[:], in_=wqkt_d[:])
            xt_n0_sb = xt_all.rearrange("p (k t) -> p k t", k=K_D)[:, :, 0:QT]
            xt_n0_dr = xt_d.rearrange("p (k t) -> p k t", k=K_D)[:, :, 0:QT]
            nc.gpsimd.dma_start(out=xt_n0_sb, in_=xt_n0_dr)
            nc.scalar.dma_start(out=wvt_all[:], in_=wvt_d[:])
            nc.scalar.dma_start(out=masks[:], in_=masks_d[:])
            nc.sync.dma_start(out=wot_all[:], in_=wot_d[:])
            for n in range(1, n_qt):
                sl = slice(n * QT, (n + 1) * QT)
                nc.gpsimd.dma_start(
                    out=xt_all.rearrange("p (k t) -> p k t", k=K_D)[:, :, sl],
                    in_=xt_d.rearrange("p (k t) -> p k t", k=K_D)[:, :, sl])

            # ones columns of v_pad (persist; set once)
            for m in range(T // P):
                ones_cols = v_pad[m].rearrange(
                    "p (h x) -> p h x", x=hd + 1)[:, :, hd:]
                nc.vector.memset(ones_cols, 1.0)

            # ---- emission units ----
            def emit_qk_group(n, m):
                ps = aps_pool.tile([P, QT], f32, name="qkps", tag="aps")
                for k in range(K_D):
                    nc.tensor.matmul(
                        ps[:],
                        lhsT=wqkt[k][:, m * P:(m + 1) * P],
                        rhs=xt[k][:, n * QT:(n + 1) * QT],
                        start=(k == 0), stop=(k == K_D - 1),
                    )
                nc.scalar.copy(qkt[m][:, n * QT:(n + 1) * QT], ps[:])

            def emit_v_group(n, i):
                m = 4 * n + i
                ps = aps_pool.tile([P, V_COLS], f32, name="vps", tag="aps")
                for k in range(K_D):
                    nc.tensor.matmul(
                        ps[:],
                        lhsT=xt[k][:, m * P:(m + 1) * P],
                        rhs=wvt[k][:],
                        start=(k == 0), stop=(k == K_D - 1),
                    )
                nc.vector.tensor_copy(
                    v_pad[m].rearrange("p (h x) -> p h x", x=hd + 1)[:, :, 0:hd],
                    ps.rearrange("p (h x) -> p h x", x=hd),
                )

            # per-(g,p) attention state carried across chunk units
            class PairState:
                pass

            def emit_chunk_front(st, g, p, j):
                """scores + exp + mask for chunk j (PV deferred one chunk)."""
                q0 = g * QT
                s0 = j * SC
                r = (s0 - q0) // SC
                w0 = max(r, 0) * SC
                sc = sc_ps.tile([P, 2 * QT], f32, name="sc", tag="sc")
                qT = qkt[p]
                kT = qkt[n_pairs + p]
                for e in range(2):
                    lo, hi = e * hd, e * hd + hd
                    nc.tensor.matmul(
                        sc[:, e * QT + w0:(e + 1) * QT],
                        lhsT=kT[lo:hi, s0:s0 + SC],
                        rhs=qT[lo:hi, q0 + w0:q0 + QT],
                        start=True, stop=True,
                    )
                u = u_pool.tile([P, 2 * QT], at_dt, name="u", tag="u")
                if w0 == 0:
                    nc.scalar.activation(
                        u[:], sc[:], mybir.ActivationFunctionType.Exp,
                        scale=inv_sqrt,
                    )
                else:
                    u_v = u.rearrange("p (e t) -> p e t", e=2)
                    sc_v = sc.rearrange("p (e t) -> p e t", e=2)
                    nc.scalar.activation(
                        u_v[:, :, w0:], sc_v[:, :, w0:],
                        mybir.ActivationFunctionType.Exp, scale=inv_sqrt,
                    )
                if r >= 0:
                    u_v = u.rearrange("p (e t) -> p e t", e=2)
                    m_v = masks[:, r * 2 * QT:(r + 1) * 2 * QT].rearrange(
                        "p (e t) -> p e t", e=2)
                    nc.vector.tensor_mul(
                        u_v[:, :, w0:w0 + SC], u_v[:, :, w0:w0 + SC],
                        m_v[:, :, w0:w0 + SC])
                st.pend.append((j, u, w0))

            def emit_chunk_pv(st, g, p):
                """PV for the oldest pending chunk."""
                j, u, w0 = st.pend.pop(0)
                n_sc = (g * QT + QT) // SC
                for e in range(2):
                    h = 2 * p + e
                    nc.tensor.matmul(
                        st.pv[0:hd + 1, e * QT + w0:(e + 1) * QT],
                        lhsT=v_pad[j][:, h * (hd + 1):(h + 1) * (hd + 1)],
                        rhs=u[:, e * QT + w0:(e + 1) * QT],
                        start=(j == 0), stop=(j == n_sc - 1),
                    )

            def emit_evict(st, g, p, final=False):
                """Drain pv psum: unnormalized outT (rows 0:64) + Z (row 64).

                final=True (last pair of the kernel, straight into OP(3)):
                split the Z chain per head so recip/broadcast/normalize
                pipeline instead of running serially on the critical path.
                """
                q0 = g * QT
                if final:
                    reps = []
                    for e in range(2):
                        zr = norm_pool.tile([1, QT], f32, name="zr",
                                            tag=f"zr{e}", bufs=1)
                        nc.vector.tensor_copy(
                            zr[0:1, :], st.pv[hd:hd + 1, e * QT:(e + 1) * QT])
                        rr = norm_pool.tile([1, QT], f32, name="rr",
                                            tag=f"rr{e}", bufs=1)
                        nc.vector.reciprocal_approx_fast(out=rr[0:1, :],
                                                         in_=zr[0:1, :])
                        rep = norm_pool.tile([P, QT], f32, name="repf",
                                             tag=f"repf{e}", bufs=1)
                        nc.gpsimd.partition_broadcast(rep[:], rr[0:1, :])
                        reps.append(rep)
                    # evicts after the Z chains are queued
                    nc.scalar.copy(attn[p][0:hd, q0:q0 + QT],
                                   st.pv[0:hd, 0:QT])
                    nc.vector.tensor_copy(
                        attn[p][hd:P, q0:q0 + QT], st.pv[0:hd, QT:2 * QT])
                    st.reps = reps
                    st.rep = None
                    return
                # head 0 evict on scalar (partition-aligned 0:64 -> 0:64)
                nc.scalar.copy(attn[p][0:hd, q0:q0 + QT], st.pv[0:hd, 0:QT])
                # Z row (both heads) to partition 0 of SBUF, then 1/Z
                zrow = norm_pool.tile([1, 2 * QT], f32, name="zrow",
                                      tag="zrow", bufs=2)
                nc.vector.tensor_copy(zrow[0:1, :], st.pv[hd:hd + 1, :])
                # head 1 evict on DVE (partition shift 0:64 -> 64:128)
                nc.vector.tensor_copy(
                    attn[p][hd:P, q0:q0 + QT], st.pv[0:hd, QT:2 * QT])
                rrow = norm_pool.tile([1, 2 * QT], f32, name="rrow",
                                      tag="rrow", bufs=2)
                nc.vector.reciprocal_approx_fast(out=rrow[0:1, :],
                                                 in_=zrow[0:1, :])
                rep = norm_pool.tile([P, 2 * QT], f32, name="rep",
                                     tag="rep", bufs=2)
                nc.gpsimd.partition_broadcast(rep[:], rrow[0:1, :])
                st.rep = rep

            def emit_norm(st, g, p):
                q0 = g * QT
                for e in range(2):
                    sl = attn[p][e * hd:(e + 1) * hd, q0:q0 + QT]
                    if st.rep is None:
                        rep_sl = st.reps[e][e * hd:(e + 1) * hd, :]
                    else:
                        rep_sl = st.rep[e * hd:(e + 1) * hd,
                                        e * QT:(e + 1) * QT]
                    nc.vector.tensor_mul(sl, sl, rep_sl)

            def emit_op_block(g, m, y_eng):
                ps = aps_pool.tile([P, QT], f32, name="yps", tag="aps")
                for k in range(C // P):
                    nc.tensor.matmul(
                        ps[:],
                        lhsT=wot[k][:, m * P:(m + 1) * P],
                        rhs=attn[k][:, g * QT:(g + 1) * QT],
                        start=(k == 0), stop=(k == C // P - 1),
                    )
                y = y_pool.tile([P, QT], at_dt, name="y", tag="y")
                if y_eng == "scalar":
                    nc.scalar.copy(y[:], ps[:])
                else:
                    nc.vector.tensor_copy(y[:], ps[:])
                nc.sync.dma_start(
                    out=yt_t[m][:, g * QT:(g + 1) * QT], in_=y[:])

            # ---- schedule ----
            # A(0): all q/k first (their DMAs land first), then v (wvt lands
            # a little later).
            for unit in [(0, 0), (0, 2), (0, 1), (0, 3)]:
                emit_qk_group(*unit)
            for i in range(4):
                emit_v_group(0, i)

            def fill_units(g):
                """PE filler for segment g: A(g+1) groups; OP blocks are
                back-loaded into the final segment (B(3) has the thinnest
                fill ratio and is exp-rate-balanced, so it needs the most
                independent PE work)."""
                units = []
                if g + 1 < n_qt:
                    a = [lambda n=g + 1, m=m: emit_qk_group(n, m)
                         for m in range(QK_ROWS // P)]
                    v = [lambda n=g + 1, i=i: emit_v_group(n, i)
                         for i in range(4)]
                    # alternate qk/v
                    for x, y in zip(a, v):
                        units += [x, y]
                ops = {1: [0], 3: [1, 2]}.get(g, [])
                for gg in ops:
                    units += [lambda gg=gg, m=m: emit_op_block(gg, m, "dve")
                              for m in range(D // P)]
                return units

            prev_norm = None  # pair-1 normalize deferred from previous seg
            for g in range(n_qt):
                n_sc = (g * QT + QT) // SC
                primary = []
                states = [PairState(), PairState()]
                if prev_norm is not None:
                    # MUST precede OP(g-1) fill units: deps follow emission
                    # order and OP reads the region this normalizes.
                    primary.append(prev_norm)
                for p in range(n_pairs):
                    st = states[p]
                    st.pend = []

                    def mk_alloc(st=st):
                        def start_pair():
                            st.pv = pv_ps.tile([hd + 1, 2 * QT], f32,
                                               name="pv", tag="pv")
                        return start_pair
                    primary.append(mk_alloc())
                    for j in range(n_sc):
                        primary.append(
                            lambda st=st, g=g, p=p, j=j:
                                emit_chunk_front(st, g, p, j))
                        if j > 0:
                            primary.append(
                                lambda st=st, g=g, p=p: emit_chunk_pv(st, g, p))
                        if p == 1 and j == 2:
                            # normalize pair 0 once its broadcast has drained
                            primary.append(
                                lambda st=states[0], g=g: emit_norm(st, g, 0))
                    primary.append(
                        lambda st=st, g=g, p=p: emit_chunk_pv(st, g, p))
                    fin = (g == n_qt - 1 and p == n_pairs - 1)
                    if fin:
                        # emit the last pair's evict AFTER all fill units so
                        # no fill work (DVE y-evicts) queues ahead of the
                        # final normalize chain.
                        final_evict = (
                            lambda st=st, g=g, p=p:
                                emit_evict(st, g, p, final=True))
                    else:
                        primary.append(
                            lambda st=st, g=g, p=p: emit_evict(st, g, p))
                # pair 1 normalize goes to the start of the next segment
                prev_norm = lambda st=states[1], g=g: emit_norm(st, g, 1)

                fills = fill_units(g)
                # interleave fills evenly between primary units
                out_plan = []
                f = 0
                for i, u in enumerate(primary):
                    out_plan.append(u)
                    want = (i + 1) * len(fills) // len(primary)
                    while f < want:
                        out_plan.append(fills[f])
                        f += 1
                for u in out_plan:
                    u()
                if g == n_qt - 1:
                    final_evict()

            # tail: last normalize + OP(3)
            prev_norm()
            for m in range(D // P):
                emit_op_block(n_qt - 1, m, "scalar" if m % 2 else "dve")

    nc.finalize()
    return nc


def _make_masks(QT=512, SC=128):
    """Doubled causal masks: [128, 4*2*QT]; block r holds the mask for
    relative offset r twice side by side (head A | head B)."""
    i = np.arange(SC)[:, None]
    j = np.arange(QT)[None, :]
    blocks = []
    for r in range(4):
        m = (i + r * SC <= j).astype(np.float32)
        blocks += [m, m]
    return np.concatenate(blocks, axis=1)


def _cast(arr, dtype_name):
    if dtype_name == "bfloat16":
        import ml_dtypes
        return np.ascontiguousarray(arr.astype(ml_dtypes.bfloat16))
    return np.ascontiguousarray(arr.astype(np.float32))


def kernel(x, Wqkv, Wout, a, b, dt):
    from concourse.bass_utils import run_bass_kernel_spmd

    x = np.asarray(x, dtype=np.float32)
    Wqkv = np.asarray(Wqkv, dtype=np.float32)
    Wout = np.asarray(Wout, dtype=np.float32)
    B, T, D = x.shape
    H, hd = N_HEADS, HEAD_DIM
    hpc = HEADS_PER_CORE
    cores_per_batch = H // hpc
    f0 = _fhn_scale(a, b, dt)

    key = (T, D, hpc, hd)
    if key not in _PROGRAM_CACHE:
        _PROGRAM_CACHE[key] = _build_program(*key)
    nc = _PROGRAM_CACHE[key]

    masks = _cast(_make_masks(), ATTN_DTYPE)

    def _pack(arr):
        """[K*128, X] -> [128, K*X]: partition-major tiles side by side."""
        K = arr.shape[0] // 128
        return np.ascontiguousarray(
            arr.reshape(K, 128, -1).transpose(1, 0, 2).reshape(128, -1))

    in_maps = []
    for c in range(N_CORES):
        bi = c // cores_per_batch
        heads = range((c % cores_per_batch) * hpc, (c % cores_per_batch) * hpc + hpc)
        q_rows = np.concatenate([np.arange(h * hd, (h + 1) * hd) for h in heads])
        xt = _pack(_cast(x[bi].T, ATTN_DTYPE))                   # (128, 8*T)
        wqk = np.concatenate([Wqkv[q_rows], Wqkv[D + q_rows]], axis=0)
        wqkt = _pack(_cast(wqk.T, ATTN_DTYPE))                   # (128, 8*512)
        wvt = _pack(_cast(Wqkv[2 * D + q_rows].T, ATTN_DTYPE))   # (128, 8*256)
        wo = (Wout[:, q_rows].astype(np.float64) * f0).astype(np.float32)
        wot = _pack(_cast(wo.T, ATTN_DTYPE))                     # (128, 2*D)
        in_maps.append({"xt": xt, "wqkt": wqkt, "wvt": wvt, "wot": wot,
                        "masks": masks})

    trace_dir = os.environ.get("KERNEL_TRACE", "")
    kwargs = {}
    if trace_dir:
        try:
            import antenv.axon_hooks  # noqa: F401
        except ImportError:
            # agent image lacks the hook module; install the ctypes shim
            try:
                import sys
                import types

                from trn_agent_boot.trn_boot import _ntff_profile_via_ctypes
                hook = _ntff_profile_via_ctypes("/opt/axon/libaxon_pjrt.so")
                mod = types.ModuleType("antenv.axon_hooks")
                mod.get_axon_ntff_profile_hook = lambda: hook
                sys.modules["antenv.axon_hooks"] = mod
            except Exception:
                trace_dir = ""
    if trace_dir:
        os.makedirs(trace_dir, exist_ok=True)
        kwargs = {"trace": True, "tmpdir": trace_dir}
    res = run_bass_kernel_spmd(nc, in_maps, list(range(N_CORES)), **kwargs)
    LAST_RUN["exec_time_ns"] = res.exec_time_ns
    LAST_RUN["profile_json"] = res.profile_json

    out = np.zeros((B, T, D), dtype=np.float32)
    for bi in range(B):
        acc = np.zeros((D, T), dtype=np.float32)
        for c in range(bi * cores_per_batch, (bi + 1) * cores_per_batch):
            acc += res.results[c]["yt"].astype(np.float32)
        out[bi] = acc.T
    return out


# revision 33
# speedup vs baseline: 1.0238x; 1.0175x over previous
# Trainium2 Bass kernel for nn_FHNTritonAttention: causal attention with an
# FHN (FitzHugh-Nagumo) gate on the attention probabilities.
#
# Math note that shapes the whole kernel: attn_energy = softmax(scores).sum(-1)
# is ~1.0 for every row (softmax rows sum to 1), so stimulus_normed == 1,
# threshold_gate == sigmoid(5), and the FHN recurrence collapses to one
# constant per run. The gate multiplies each probability row by a constant c
# and the subsequent renormalization divides it back out:
#   p'' = p*c / (c*S + 1e-8) = p / (S + 1e-8/c),  S = row sum ~= 1.
# So the entire FHN block reduces to scaling the output by
# f0 = 1/(1 + 1e-8/c0), computed on host from (a, b, dt) and folded into Wout.
# The deviations this ignores are O(1e-7) relative — far below fp32 matmul
# noise of the reference itself.
#
# Device kernel (SPMD over 8 cores; core = (batch, 4-head group)):
#   The whole kernel is emitted as ONE software-pipelined stream so the PE
#   never stalls on the softmax (scalar engine) dependency chain:
#     - A(n) blocks: qkT = Wqk_slice @ x.T (feature-on-partition layout) and
#       v (natural layout + ones column for the softmax denominator Z).
#     - B(g) chunks: per head pair, per 128-deep s chunk: scoresT for both
#       heads into one [128,1024] psum, one strided exp (scalar), causal-mask
#       multiply on diagonal chunks (gpsimd), PV accumulate (with ones column
#       producing Z). PV for chunk j is emitted one chunk late so the PE
#       in-order queue never waits on exp.
#     - OP(g) blocks: out-projection per q tile, evict + DMA out.
#   Schedule: A(0); then for each g: B(g) chunks interleaved with A(g+1) and
#   OP(g-1) blocks as PE filler. Evictions/normalize are spread across
#   scalar (head-0 evict, qk evict), DVE (recip, head-1 evict, normalize,
#   v/y evict) and gpsimd (1/Z partition-broadcast, masks) so no engine
#   exceeds the PE's total work.
#
# Matmuls run in bf16 (inputs pre-cast on host; fp32 PSUM accumulation).

import math
import os

import numpy as np

N_HEADS = 16
HEAD_DIM = 64
THRESHOLD = 0.5
TAU = 12.5
N_FHN_STEPS = 2

N_CORES = 8
HEADS_PER_CORE = 4  # cores 0-3 -> batch 0, cores 4-7 -> batch 1

ATTN_DTYPE = os.environ.get("KERNEL_ATTN_DTYPE", "bfloat16")

LAST_RUN = {}  # filled with exec_time_ns etc. when KERNEL_TRACE is set

_PROGRAM_CACHE = {}


def _fhn_scale(a, b, dt):
    """Host-side replica of the reference's gate math at attn_energy == 1."""
    a = float(a)
    b = float(b)
    dt = float(dt)
    sig5 = 1.0 / (1.0 + math.exp(-(1.0 - THRESHOLD) * 10.0))
    i0 = 1.0 * (0.1 + 0.9 * sig5)
    v = 0.0
    w = 0.0
    for _ in range(N_FHN_STEPS):
        v = v + dt * (v - v**3 / 3.0 - w + i0)
        w = (w + (dt / TAU) * (v + a)) / (1.0 + (dt / TAU) * b)
    gate = 1.0 / (1.0 + math.exp(-v))
    c0 = 0.5 + 0.5 * gate
    return c0 / (c0 + 1e-8)


def _build_program(T, D, H_per_core, hd):
    import concourse.mybir as mybir
    import concourse.tile as tile
    from concourse import bacc

    f32 = mybir.dt.float32
    at_dt = getattr(mybir.dt, ATTN_DTYPE)
    P = 128
    QT = 512   # q tile width (free dim of score/PV matmuls)
    SC = 128   # s chunk depth (contraction of PV, partitions of scoresT)
    K_D = D // P
    QK_ROWS = 2 * H_per_core * hd
    V_COLS = H_per_core * hd
    C = H_per_core * hd
    n_qt = T // QT
    n_pairs = H_per_core // 2
    inv_sqrt = 1.0 / math.sqrt(hd)

    nc = bacc.Bacc("TRN2", target_bir_lowering=False, debug=False,
                   num_devices=N_CORES)

    xt_d = nc.declare_dram_parameter("xt", [D, T], at_dt, isOutput=False)
    wqkt_d = nc.declare_dram_parameter("wqkt", [D, QK_ROWS], at_dt, isOutput=False)
    wvt_d = nc.declare_dram_parameter("wvt", [D, V_COLS], at_dt, isOutput=False)
    wot_d = nc.declare_dram_parameter("wot", [C, D], at_dt, isOutput=False)
    masks_d = nc.declare_dram_parameter("masks", [P, 4 * 2 * QT], at_dt,
                                        isOutput=False)
    yt_d = nc.declare_dram_parameter("yt", [D, T], at_dt, isOutput=True)

    xt_t = xt_d.rearrange("(a p) t -> a p t", p=P)
    wqkt_t = wqkt_d.rearrange("(a p) m -> a p m", p=P)
    wvt_t = wvt_d.rearrange("(a p) m -> a p m", p=P)
    wot_t = wot_d.rearrange("(a p) m -> a p m", p=P)
    yt_t = yt_d.rearrange("(a p) t -> a p t", p=P)

    with nc.allow_low_precision(reason="bf16/f32r compute is intentional"), \
            tile.TileContext(nc) as tc:
        with (
            tc.tile_pool(name="persist", bufs=1) as persist,
            tc.tile_pool(name="aps_pool", bufs=2, space="PSUM") as aps_pool,
            tc.tile_pool(name="sc_ps", bufs=2, space="PSUM") as sc_ps,
            tc.tile_pool(name="pv_ps", bufs=1, space="PSUM") as pv_ps,
            tc.tile_pool(name="u_sb", bufs=6) as u_pool,
            tc.tile_pool(name="norm", bufs=1) as norm_pool,
            tc.tile_pool(name="y_sb", bufs=6) as y_pool,
        ):
            # ---- persistent SBUF tiles ----
            xt = [persist.tile([P, T], at_dt, name=f"xt{i}", tag=f"xt{i}")
                  for i in range(K_D)]
            wqkt = [persist.tile([P, QK_ROWS], at_dt, name=f"wqkt{i}",
                                 tag=f"wqkt{i}") for i in range(K_D)]
            wvt = [persist.tile([P, V_COLS], at_dt, name=f"wvt{i}",
                                tag=f"wvt{i}") for i in range(K_D)]
            masks = persist.tile([P, 8 * QT], at_dt, name="masks", tag="masks")
            wot = [persist.tile([P, D], at_dt, name=f"wot{i}", tag=f"wot{i}")
                   for i in range(C // P)]
            qkt = [persist.tile([P, T], at_dt, name=f"qkt{m}", tag=f"qkt{m}")
                   for m in range(QK_ROWS // P)]
            v_pad = [persist.tile([P, H_per_core * (hd + 1)], at_dt,
                                  name=f"vp{m}", tag=f"vp{m}")
                     for m in range(T // P)]
            attn = [persist.tile([P, T], at_dt, name=f"attn{p}", tag=f"attn{p}")
                    for p in range(n_pairs)]

            # ---- input DMAs: submission spread across engine queues so the
            # descriptor generation (~0.6us per dma_start) does not serialize
            # the input stream on one sequencer.
            for i in range(K_D):
                (nc.sync if i % 2 else nc.scalar).dma_start(
                    out=wqkt[i][:], in_=wqkt_t[i])
                nc.gpsimd.dma_start(out=xt[i][:, 0:QT], in_=xt_t[i][:, 0:QT])
                (nc.scalar if i % 2 else nc.sync).dma_start(
                    out=wvt[i][:], in_=wvt_t[i])
            nc.scalar.dma_start(out=masks[:], in_=masks_d[:])
            for i in range(C // P):
                nc.sync.dma_start(out=wot[i][:], in_=wot_t[i])
            for i in range(K_D):
                (nc.sync if i % 2 else nc.gpsimd).dma_start(
                    out=xt[i][:, QT:], in_=xt_t[i][:, QT:])

            # ones columns of v_pad (persist; set once)
            for m in range(T // P):
                ones_cols = v_pad[m].rearrange(
                    "p (h x) -> p h x", x=hd + 1)[:, :, hd:]
                nc.vector.memset(ones_cols, 1.0)

            # ---- emission units ----
            def emit_qk_group(n, m):
                ps = aps_pool.tile([P, QT], f32, name="qkps", tag="aps")
                for k in range(K_D):
                    nc.tensor.matmul(
                        ps[:],
                        lhsT=wqkt[k][:, m * P:(m + 1) * P],
                        rhs=xt[k][:, n * QT:(n + 1) * QT],
                        start=(k == 0), stop=(k == K_D - 1),
                    )
                nc.scalar.copy(qkt[m][:, n * QT:(n + 1) * QT], ps[:])

            def emit_v_group(n, i):
                m = 4 * n + i
                ps = aps_pool.tile([P, V_COLS], f32, name="vps", tag="aps")
                for k in range(K_D):
                    nc.tensor.matmul(
                        ps[:],
                        lhsT=xt[k][:, m * P:(m + 1) * P],
                        rhs=wvt[k][:],
                        start=(k == 0), stop=(k == K_D - 1),
                    )
                nc.vector.tensor_copy(
                    v_pad[m].rearrange("p (h x) -> p h x", x=hd + 1)[:, :, 0:hd],
                    ps.rearrange("p (h x) -> p h x", x=hd),
                )

            # per-(g,p) attention state carried across chunk units
            class PairState:
                pass

            def emit_chunk_front(st, g, p, j):
                """scores + exp + mask for chunk j (PV deferred one chunk)."""
                q0 = g * QT
                s0 = j * SC
                r = (s0 - q0) // SC
                w0 = max(r, 0) * SC
                sc = sc_ps.tile([P, 2 * QT], f32, name="sc", tag="sc")
                qT = qkt[p]
                kT = qkt[n_pairs + p]
                for e in range(2):
                    lo, hi = e * hd, e * hd + hd
                    nc.tensor.matmul(
                        sc[:, e * QT + w0:(e + 1) * QT],
                        lhsT=kT[lo:hi, s0:s0 + SC],
                        rhs=qT[lo:hi, q0 + w0:q0 + QT],
                        start=True, stop=True,
                    )
                u = u_pool.tile([P, 2 * QT], at_dt, name="u", tag="u")
                if w0 == 0:
                    nc.scalar.activation(
                        u[:], sc[:], mybir.ActivationFunctionType.Exp,
                        scale=inv_sqrt,
                    )
                else:
                    u_v = u.rearrange("p (e t) -> p e t", e=2)
                    sc_v = sc.rearrange("p (e t) -> p e t", e=2)
                    nc.scalar.activation(
                        u_v[:, :, w0:], sc_v[:, :, w0:],
                        mybir.ActivationFunctionType.Exp, scale=inv_sqrt,
                    )
                if r >= 0:
                    u_v = u.rearrange("p (e t) -> p e t", e=2)
                    m_v = masks[:, r * 2 * QT:(r + 1) * 2 * QT].rearrange(
                        "p (e t) -> p e t", e=2)
                    nc.vector.tensor_mul(
                        u_v[:, :, w0:w0 + SC], u_v[:, :, w0:w0 + SC],
                        m_v[:, :, w0:w0 + SC])
                st.pend.append((j, u, w0))

            def emit_chunk_pv(st, g, p):
                """PV for the oldest pending chunk."""
                j, u, w0 = st.pend.pop(0)
                n_sc = (g * QT + QT) // SC
                for e in range(2):
                    h = 2 * p + e
                    nc.tensor.matmul(
                        st.pv[0:hd + 1, e * QT + w0:(e + 1) * QT],
                        lhsT=v_pad[j][:, h * (hd + 1):(h + 1) * (hd + 1)],
                        rhs=u[:, e * QT + w0:(e + 1) * QT],
                        start=(j == 0), stop=(j == n_sc - 1),
                    )

            def emit_evict(st, g, p, final=False):
                """Drain pv psum: unnormalized outT (rows 0:64) + Z (row 64).

                final=True (last pair of the kernel, straight into OP(3)):
                split the Z chain per head so recip/broadcast/normalize
                pipeline instead of running serially on the critical path.
                """
                q0 = g * QT
                if final:
                    reps = []
                    for e in range(2):
                        zr = norm_pool.tile([1, QT], f32, name="zr",
                                            tag=f"zr{e}", bufs=1)
                        nc.vector.tensor_copy(
                            zr[0:1, :], st.pv[hd:hd + 1, e * QT:(e + 1) * QT])
                        rr = norm_pool.tile([1, QT], f32, name="rr",
                                            tag=f"rr{e}", bufs=1)
                        nc.vector.reciprocal_approx_fast(out=rr[0:1, :],
                                                         in_=zr[0:1, :])
                        rep = norm_pool.tile([P, QT], f32, name="repf",
                                             tag=f"repf{e}", bufs=1)
                        nc.gpsimd.partition_broadcast(rep[:], rr[0:1, :])
                        reps.append(rep)
                    # evicts after the Z chains are queued
                    nc.scalar.copy(attn[p][0:hd, q0:q0 + QT],
                                   st.pv[0:hd, 0:QT])
                    nc.vector.tensor_copy(
                        attn[p][hd:P, q0:q0 + QT], st.pv[0:hd, QT:2 * QT])
                    st.reps = reps
                    st.rep = None
                    return
                # head 0 evict on scalar (partition-aligned 0:64 -> 0:64)
                nc.scalar.copy(attn[p][0:hd, q0:q0 + QT], st.pv[0:hd, 0:QT])
                # Z row (both heads) to partition 0 of SBUF, then 1/Z
                zrow = norm_pool.tile([1, 2 * QT], f32, name="zrow",
                                      tag="zrow", bufs=2)
                nc.vector.tensor_copy(zrow[0:1, :], st.pv[hd:hd + 1, :])
                # head 1 evict on DVE (partition shift 0:64 -> 64:128)
                nc.vector.tensor_copy(
                    attn[p][hd:P, q0:q0 + QT], st.pv[0:hd, QT:2 * QT])
                rrow = norm_pool.tile([1, 2 * QT], f32, name="rrow",
                                      tag="rrow", bufs=2)
                nc.vector.reciprocal_approx_fast(out=rrow[0:1, :],
                                                 in_=zrow[0:1, :])
                rep = norm_pool.tile([P, 2 * QT], f32, name="rep",
                                     tag="rep", bufs=2)
                nc.gpsimd.partition_broadcast(rep[:], rrow[0:1, :])
                st.rep = rep

            def emit_norm(st, g, p):
                q0 = g * QT
                for e in range(2):
                    sl = attn[p][e * hd:(e + 1) * hd, q0:q0 + QT]
                    if st.rep is None:
                        rep_sl = st.reps[e][e * hd:(e + 1) * hd, :]
                    else:
                        rep_sl = st.rep[e * hd:(e + 1) * hd,
                                        e * QT:(e + 1) * QT]
                    nc.vector.tensor_mul(sl, sl, rep_sl)

            def emit_op_block(g, m, y_eng):
                ps = aps_pool.tile([P, QT], f32, name="yps", tag="aps")
                for k in range(C // P):
                    nc.tensor.matmul(
                        ps[:],
                        lhsT=wot[k][:, m * P:(m + 1) * P],
                        rhs=attn[k][:, g * QT:(g + 1) * QT],
                        start=(k == 0), stop=(k == C // P - 1),
                    )
                y = y_pool.tile([P, QT], at_dt, name="y", tag="y")
                if y_eng == "scalar":
                    nc.scalar.copy(y[:], ps[:])
                else:
                    nc.vector.tensor_copy(y[:], ps[:])
                nc.sync.dma_start(
                    out=yt_t[m][:, g * QT:(g + 1) * QT], in_=y[:])

            # ---- schedule ----
            # A(0): all q/k first (their DMAs land first), then v (wvt lands
            # a little later).
            for unit in [(0, 0), (0, 2), (0, 1), (0, 3)]:
                emit_qk_group(*unit)
            for i in range(4):
                emit_v_group(0, i)

            def fill_units(g):
                """PE filler for segment g: A(g+1) groups; OP blocks are
                back-loaded into the final segment (B(3) has the thinnest
                fill ratio and is exp-rate-balanced, so it needs the most
                independent PE work)."""
                units = []
                if g + 1 < n_qt:
                    a = [lambda n=g + 1, m=m: emit_qk_group(n, m)
                         for m in range(QK_ROWS // P)]
                    v = [lambda n=g + 1, i=i: emit_v_group(n, i)
                         for i in range(4)]
                    # alternate qk/v
                    for x, y in zip(a, v):
                        units += [x, y]
                ops = {1: [0], 3: [1, 2]}.get(g, [])
                for gg in ops:
                    units += [lambda gg=gg, m=m: emit_op_block(gg, m, "dve")
                              for m in range(D // P)]
                return units

            prev_norm = None  # pair-1 normalize deferred from previous seg
            for g in range(n_qt):
                n_sc = (g * QT + QT) // SC
                primary = []
                states = [PairState(), PairState()]
                if prev_norm is not None:
                    # MUST precede OP(g-1) fill units: deps follow emission
                    # order and OP reads the region this normalizes.
                    primary.append(prev_norm)
                for p in range(n_pairs):
                    st = states[p]
                    st.pend = []

                    def mk_alloc(st=st):
                        def start_pair():
                            st.pv = pv_ps.tile([hd + 1, 2 * QT], f32,
                                               name="pv", tag="pv")
                        return start_pair
                    primary.append(mk_alloc())
                    for j in range(n_sc):
                        primary.append(
                            lambda st=st, g=g, p=p, j=j:
                                emit_chunk_front(st, g, p, j))
                        if j > 0:
                            primary.append(
                                lambda st=st, g=g, p=p: emit_chunk_pv(st, g, p))
                        if p == 1 and j == 2:
                            # normalize pair 0 once its broadcast has drained
                            primary.append(
                                lambda st=states[0], g=g: emit_norm(st, g, 0))
                    primary.append(
                        lambda st=st, g=g, p=p: emit_chunk_pv(st, g, p))
                    fin = (g == n_qt - 1 and p == n_pairs - 1)
                    if fin:
                        # emit the last pair's evict AFTER all fill units so
                        # no fill work (DVE y-evicts) queues ahead of the
                        # final normalize chain.
                        final_evict = (
                            lambda st=st, g=g, p=p:
                                emit_evict(st, g, p, final=True))
                    else:
                        primary.append(
                            lambda st=st, g=g, p=p: emit_evict(st, g, p))
                # pair 1 normalize goes to the start of the next segment
                prev_norm = lambda st=states[1], g=g: emit_norm(st, g, 1)

                fills = fill_units(g)
                # interleave fills evenly between primary units
                out_plan = []
                f = 0
                for i, u in enumerate(primary):
                    out_plan.append(u)
                    want = (i + 1) * len(fills) // len(primary)
                    while f < want:
                        out_plan.append(fills[f])
                        f += 1
                for u in out_plan:
                    u()
                if g == n_qt - 1:
                    final_evict()

            # tail: last normalize + OP(3)
            prev_norm()
            for m in range(D // P):
                emit_op_block(n_qt - 1, m, "scalar" if m % 2 else "dve")

    nc.finalize()
    return nc


def _make_masks(QT=512, SC=128):
    """Doubled causal masks: [128, 4*2*QT]; block r holds the mask for
    relative offset r twice side by side (head A | head B)."""
    i = np.arange(SC)[:, None]
    j = np.arange(QT)[None, :]
    blocks = []
    for r in range(4):
        m = (i + r * SC <= j).astype(np.float32)
        blocks += [m, m]
    return np.concatenate(blocks, axis=1)


def _cast(arr, dtype_name):
    if dtype_name == "bfloat16":
        import ml_dtypes
        return np.ascontiguousarray(arr.astype(ml_dtypes.bfloat16))
    return np.ascontiguousarray(arr.astype(np.float32))


def kernel(x, Wqkv, Wout, a, b, dt):
    from concourse.bass_utils import run_bass_kernel_spmd

    x = np.asarray(x, dtype=np.float32)
    Wqkv = np.asarray(Wqkv, dtype=np.float32)
    Wout = np.asarray(Wout, dtype=np.float32)
    B, T, D = x.shape
    H, hd = N_HEADS, HEAD_DIM
    hpc = HEADS_PER_CORE
    cores_per_batch = H // hpc
    f0 = _fhn_scale(a, b, dt)

    key = (T, D, hpc, hd)
    if key not in _PROGRAM_CACHE:
        _PROGRAM_CACHE[key] = _build_program(*key)
    nc = _PROGRAM_CACHE[key]

    masks = _cast(_make_masks(), ATTN_DTYPE)
    in_maps = []
    for c in range(N_CORES):
        bi = c // cores_per_batch
        heads = range((c % cores_per_batch) * hpc, (c % cores_per_batch) * hpc + hpc)
        q_rows = np.concatenate([np.arange(h * hd, (h + 1) * hd) for h in heads])
        xt = _cast(x[bi].T, ATTN_DTYPE)                          # (D, T)
        wqk = np.concatenate([Wqkv[q_rows], Wqkv[D + q_rows]], axis=0)
        wqkt = _cast(wqk.T, ATTN_DTYPE)                          # (D, 2*hpc*hd)
        wvt = _cast(Wqkv[2 * D + q_rows].T, ATTN_DTYPE)          # (D, hpc*hd)
        wo = (Wout[:, q_rows].astype(np.float64) * f0).astype(np.float32)
        wot = _cast(wo.T, ATTN_DTYPE)                            # (hpc*hd, D)
        in_maps.append({"xt": xt, "wqkt": wqkt, "wvt": wvt, "wot": wot,
                        "masks": masks})

    trace_dir = os.environ.get("KERNEL_TRACE", "")
    kwargs = {}
    if trace_dir:
        try:
            import antenv.axon_hooks  # noqa: F401
        except ImportError:
            # agent image lacks the hook module; install the ctypes shim
            try:
                import sys
                import types

                from trn_agent_boot.trn_boot import _ntff_profile_via_ctypes
                hook = _ntff_profile_via_ctypes("/opt/axon/libaxon_pjrt.so")
                mod = types.ModuleType("antenv.axon_hooks")
                mod.get_axon_ntff_profile_hook = lambda: hook
                sys.modules["antenv.axon_hooks"] = mod
            except Exception:
                trace_dir = ""
    if trace_dir:
        os.makedirs(trace_dir, exist_ok=True)
        kwargs = {"trace": True, "tmpdir": trace_dir}
    res = run_bass_kernel_spmd(nc, in_maps, list(range(N_CORES)), **kwargs)
    LAST_RUN["exec_time_ns"] = res.exec_time_ns
    LAST_RUN["profile_json"] = res.profile_json

    out = np.zeros((B, T, D), dtype=np.float32)
    for bi in range(B):
        acc = np.zeros((D, T), dtype=np.float32)
        for c in range(bi * cores_per_batch, (bi + 1) * cores_per_batch):
            acc += res.results[c]["yt"].astype(np.float32)
        out[bi] = acc.T
    return out
